# revision 1
# baseline (speedup 1.0000x reference)
"""EvolveGCN (2-layer GCN + GRU weight evolution) on 8 Trainium2 NeuronCores.

Sharding:
  - Nodes/edges sharded by destination across the 8 cores (graph partitioning).
  - SpMM aggregation per core: fp32 source rows are fetched with dma_gather
    (int16 indices; the mod-4 residue-class trick keeps every index < 32768)
    into 128-slot chunks, then reduced on the TensorEngine with 0/1
    "staircase" segment matrices (built on-device with is_equal) into PSUM
    column windows.  Destination nodes are bin-packed (16 columns / 512 slots
    per bin, one residue class per column position, <=128 slots per source
    class) so the instruction stream is identical on every core (SPMD); all
    data-dependent structure lives in index tables.
  - GRU: the 12288x4096 matrices are tensor-parallel-sharded along the gate
    dim (1536 rows per core); the host pre-transposes them so weight tiles
    stream directly as lhsT (cast to bf16 in-flight).  Gate slices are
    combined with one small AllGather.
  - h1 is exchanged with per-tile AllGathers that overlap the remaining
    layer-1 compute.
"""
import sys
sys.path.insert(0, '/opt/trn_rl_repo')
import numpy as np

N_CORES = 8
D = 64
VARIANT = {"no_cc": False, "one_cc": False, "no_gather": False, "no_pe": False,
           "g_idx": 1024, "g_q": 1, "scratch": 16384, "sp": True}
BIN_COLS = 16              # columns per bin (staircase width)
BIN_SLOTS = 512            # slots per bin (128 per class)
BINS_PER_TILE = 32         # -> 512 columns, 16384 slots per tile
CPT = 128                  # chunks per tile (4 classes x 32 bins)
COLS_PER_TILE = BIN_COLS * BINS_PER_TILE


# ----------------------------------------------------------------- host prep
def _pack_core(deg, cls_cnt):
    """Assign npc nodes to bins: 4 nodes per residue class per bin, per-source
    -class slot load <= 128.  Returns (nbins, col_of_node)."""
    npc = len(deg)
    res = np.arange(npc) % 4
    need_nodes = max(int(np.ceil((np.bincount(res, minlength=4)).max() / 4)), 1)
    need_cap = int(np.ceil(cls_cnt.sum(0).max() / 125))
    nbins = max(need_nodes, need_cap)
    # round up to a whole tile so the shared T doesn't inflate
    nbins = ((nbins + BINS_PER_TILE - 1) // BINS_PER_TILE) * BINS_PER_TILE
    while True:
        node_bin = np.full(npc, -1, np.int64)
        node_rpos = np.full(npc, -1, np.int64)
        ok = True
        for r in range(4):
            nodes_r = np.flatnonzero(res == r)
            order = nodes_r[np.argsort(-deg[nodes_r], kind="stable")]
            for rnd in range(0, len(order), nbins):
                seg = order[rnd:rnd + nbins]
                ids = np.arange(len(seg))
                if (rnd // nbins) % 2:
                    ids = nbins - 1 - ids
                node_bin[seg] = ids
                node_rpos[seg] = rnd // nbins
            if len(order) > 4 * nbins:
                ok = False
        if ok:
            # per-bin per-class loads
            loads = np.zeros((nbins, 4), np.int64)
            np.add.at(loads, node_bin, cls_cnt)
            cnts = np.zeros((nbins, 4), np.int64)     # residue counts
            np.add.at(cnts, (node_bin, res), 1)
            for _ in range(40000):
                viol = np.flatnonzero((loads > 128).any(1))
                if len(viol) == 0:
                    break
                b = viol[0]
                k = int(np.argmax(loads[b]))
                members = np.flatnonzero(node_bin == b)
                m = members[np.argmax(cls_cnt[members, k])]
                r = res[m]
                room = ((loads + cls_cnt[m] <= 128).all(1)
                        & (cnts[:, r] < 4))
                room[b] = False
                cand = np.flatnonzero(room)
                if len(cand) == 0:
                    ok = False
                    break
                tgt = cand[np.argmin(loads[cand].max(1))]
                loads[b] -= cls_cnt[m]; cnts[b, r] -= 1
                loads[tgt] += cls_cnt[m]; cnts[tgt, r] += 1
                node_bin[m] = tgt
            else:
                ok = False
        if ok and len(np.flatnonzero((loads > 128).any(1))) == 0:
            # recompute residue positions after repair
            col = np.full(npc, -1, np.int64)
            for r in range(4):
                nodes_r = np.flatnonzero(res == r)
                order = nodes_r[np.argsort(node_bin[nodes_r], kind="stable")]
                b_sorted = node_bin[order]
                start = np.searchsorted(b_sorted, np.arange(nbins))
                within = np.arange(len(order)) - start[b_sorted]
                assert within.max() <= 3
                col[order] = node_bin[order] * BIN_COLS + within * 4 + r
            return nbins, col
        nbins += BINS_PER_TILE


def wrap16(v, pad_to=None):
    v = np.asarray(v, np.int64)
    if pad_to is not None and len(v) < pad_to:
        v = np.concatenate([v, np.zeros(pad_to - len(v), np.int64)])
    assert len(v) % 16 == 0
    w = v.reshape(-1, 16).T.astype(np.int16)
    return np.tile(w, (8, 1))


def preprocess(src, dst, n_nodes):
    npc = n_nodes // N_CORES
    deg_out = np.bincount(src, minlength=n_nodes).astype(np.int64)
    deg_in = np.bincount(dst, minlength=n_nodes).astype(np.int64)
    core_of = dst // npc
    scls = src % 4

    cols = np.empty(n_nodes, np.int64)
    nbins_c = []
    for c in range(N_CORES):
        sel = core_of == c
        dl = dst[sel] - c * npc
        cc = np.zeros((npc, 4), np.int64)
        np.add.at(cc, (dl, scls[sel]), 1)
        nb, col = _pack_core(deg_in[c * npc:(c + 1) * npc], cc)
        nbins_c.append(nb)
        cols[c * npc:(c + 1) * npc] = col
    T = (max(nbins_c) + BINS_PER_TILE - 1) // BINS_PER_TILE
    NBINS = T * BINS_PER_TILE
    C = T * CPT
    NCOL = NBINS * BIN_COLS

    tile_of = cols // COLS_PER_TILE
    cin = cols % COLS_PER_TILE
    pos = (tile_of * (N_CORES * COLS_PER_TILE)
           + (np.arange(n_nodes) // npc) * COLS_PER_TILE + cin)
    R_H = T * N_CORES * COLS_PER_TILE
    assert R_H // 4 < 32768, f"h1s rows {R_H} too large for int16/4"

    cores = []
    for c in range(N_CORES):
        col = cols[c * npc:(c + 1) * npc]
        sel = core_of == c
        e_src = src[sel]
        e_col = col[dst[sel] - c * npc]          # global col in [0, NCOL)
        e_cls = e_src % 4
        ebin = e_col // BIN_COLS
        # order edges by (bin, class)
        key = ebin * 4 + e_cls
        order = np.argsort(key, kind="stable")
        e_src = e_src[order]; e_col = e_col[order]
        key = key[order]
        e_cls = e_src % 4
        ebin = e_col // BIN_COLS
        start = np.searchsorted(key, np.arange(NBINS * 4))
        counts = np.diff(np.append(start, len(key)))
        assert counts.max() <= 128
        within = np.arange(len(key)) - start[key]
        # slot id: tile t, class k, bin-local g, position p
        t_e = ebin // BINS_PER_TILE
        g_e = ebin % BINS_PER_TILE
        slot = ((t_e * 4 + e_cls) * BINS_PER_TILE + g_e) * 128 + within

        NSLOT = T * 4 * BINS_PER_TILE * 128
        idx1 = np.zeros(NSLOT, np.int64)
        idx1[slot] = e_src // 4
        idx2 = np.zeros(NSLOT, np.int64)
        idx2[slot] = pos[e_src] // 4
        relc = np.full(NSLOT, 16.0, np.float32)          # pads match nothing
        relc[slot] = (e_col % BIN_COLS).astype(np.float32)

        # device layouts -------------------------------------------------
        # gather call (t, k): 4096 idxs -> [128, T*4*256] int16 wrapped
        idx1_d = wrap16(idx1.reshape(T * 4, 4096)
                        .reshape(-1)).reshape(128, T * 4 * 256)
        idx2_d = wrap16(idx2.reshape(T * 4, 4096)
                        .reshape(-1)).reshape(128, T * 4 * 256)
        # relcol per chunk: slot (p, chunkcol) -> [128, C] f32
        relc_d = np.ascontiguousarray(
            relc.reshape(C, 128).T.astype(np.float32))
        # scatter tables per tile: psum col -> local row
        node_at = np.full(NCOL, npc, np.int64)
        node_at[col] = np.arange(npc)
        scat_d = wrap16(node_at.reshape(T, COLS_PER_TILE)
                        .reshape(-1)).reshape(128, T * 32)
        cin_rows = np.arange(NCOL) % COLS_PER_TILE
        h1scat_d = wrap16(cin_rows.reshape(-1)).reshape(128, T * 32)

        dli = np.ones(NCOL, np.float32)
        dli[col] = np.maximum(deg_in[c * npc:(c + 1) * npc], 1)
        dlo = np.ones(NCOL, np.float32)
        dlo[col] = np.maximum(deg_out[c * npc:(c + 1) * npc], 1)
        deg_in_row = dli.reshape(1, NCOL).copy()
        deg_out_blk = (dlo.reshape(T, 4, 128).transpose(2, 0, 1)
                       .reshape(128, T * 4).copy())
        cores.append(dict(idx1=idx1_d, idx2=idx2_d, relc=relc_d, scat=scat_d,
                          h1scat=h1scat_d, deg_in_row=deg_in_row,
                          deg_out_blk=deg_out_blk))

    NT_X = (n_nodes + 127) // 128
    deg_out_x = np.ones(NT_X * 128, np.float32)
    deg_out_x[:n_nodes] = np.maximum(deg_out, 1)
    deg_out_x = np.ascontiguousarray(deg_out_x.reshape(NT_X, 128).T)
    return dict(T=T, C=C, cores=cores, deg_out_x=deg_out_x, NT_X=NT_X,
                pos=pos, cols=cols)


# ------------------------------------------------------------ device builder
def build_kernel(n_nodes, T, NT_X, hdim, repeat=1):
    import concourse.bass as bass
    import concourse.bacc as bacc
    import concourse.mybir as mybir
    import concourse.tile as tile
    from concourse.masks import make_identity

    npc = n_nodes // N_CORES
    C = T * CPT
    R_H = T * N_CORES * COLS_PER_TILE
    KCH = hdim // 128
    gpc = hdim // N_CORES
    MT = gpc // 128
    f32, bf16, i16, i32 = (mybir.dt.float32, mybir.dt.bfloat16,
                           mybir.dt.int16, mybir.dt.int32)
    AF = mybir.ActivationFunctionType

    nc = bacc.Bacc(dynamic_dma_scratch_size=VARIANT["scratch"],
                   num_swdge_queues=VARIANT["g_q"])
    emb = nc.dram_tensor("emb", [n_nodes, D], f32, kind="ExternalInput")
    wihT = nc.dram_tensor("wihT", [hdim, 3 * gpc], f32, kind="ExternalInput")
    whhT = nc.dram_tensor("whhT", [hdim, 3 * gpc], f32, kind="ExternalInput")
    xg = nc.dram_tensor("xg", [128, KCH, 2], f32, kind="ExternalInput")
    hg = nc.dram_tensor("hg", [128, KCH, 2], f32, kind="ExternalInput")
    hl = nc.dram_tensor("hl", [128, MT, 2], f32, kind="ExternalInput")
    brz = nc.dram_tensor("brz", [128, 2 * MT], f32, kind="ExternalInput")
    bnih = nc.dram_tensor("bnih", [128, MT], f32, kind="ExternalInput")
    bnhh = nc.dram_tensor("bnhh", [128, MT], f32, kind="ExternalInput")
    gb1 = nc.dram_tensor("gb1", [1, D], f32, kind="ExternalInput")
    gb2 = nc.dram_tensor("gb2", [1, D], f32, kind="ExternalInput")
    idx1_t = nc.dram_tensor("idx1", [128, T * 4 * 256], i16, kind="ExternalInput")
    idx2_t = nc.dram_tensor("idx2", [128, T * 4 * 256], i16, kind="ExternalInput")
    relc_t = nc.dram_tensor("relc", [128, C], f32, kind="ExternalInput")
    iota_t = nc.dram_tensor("iota", [128, BIN_COLS], f32, kind="ExternalInput")
    scat_t = nc.dram_tensor("scat", [128, T * 32], i16, kind="ExternalInput")
    h1scat_t = nc.dram_tensor("h1scat", [128, T * 32], i16, kind="ExternalInput")
    degin_t = nc.dram_tensor("degin", [1, T * COLS_PER_TILE], f32, kind="ExternalInput")
    degout_t = nc.dram_tensor("degout", [128, T * 4], f32, kind="ExternalInput")
    degoutx_t = nc.dram_tensor("degoutx", [128, NT_X], f32, kind="ExternalInput")
    out_t = nc.dram_tensor("out", [npc + 1, D], f32, kind="ExternalOutput")

    xs = nc.dram_tensor("xs", [n_nodes, D], f32)
    h1s = nc.dram_tensor("h1s", [R_H, D], f32, addr_space="Shared")
    wbounce = nc.dram_tensor("wbounce", [gpc, 2], f32)
    wfull = nc.dram_tensor("wfull", [hdim, 2], f32, addr_space="Shared")
    h1b = [nc.dram_tensor(f"h1b{t}", [COLS_PER_TILE, D], f32) for t in range(T)]
    RG = [list(range(N_CORES))]

    with tile.TileContext(nc) as tc, \
            tc.tile_pool(name="const", bufs=1) as const_pool:
      ident = const_pool.tile([64, 64], f32)
      make_identity(nc, ident[:])
      iota_sb = const_pool.tile([128, BIN_COLS], f32)
      nc.sync.dma_start(out=iota_sb[:], in_=iota_t[:])
      for _rep in range(repeat):
        with (
            tc.tile_pool(name="gru_w", bufs=3) as gru_pool,
            tc.tile_pool(name="gru_ps", bufs=1, space="PSUM") as gru_ps_pool,
            tc.tile_pool(name="gru_sb", bufs=1) as gru_sb_pool,
        ):
            # ---------------- GRU ----------------------------------------
            xg_sb = gru_sb_pool.tile([128, KCH, 2], f32)
            hg_sb = gru_sb_pool.tile([128, KCH, 2], f32)
            nc.sync.dma_start(out=xg_sb[:], in_=xg[:])
            nc.sync.dma_start(out=hg_sb[:], in_=hg[:])

            ps_rz = gru_ps_pool.tile([128, 4 * MT], f32)
            ps_in = gru_ps_pool.tile([128, 2 * MT], f32)
            ps_hn = gru_ps_pool.tile([128, 2 * MT], f32)
            for k in range(KCH):
                wih_k = gru_pool.tile([128, 3 * gpc], f32, tag="wih")
                whh_k = gru_pool.tile([128, 3 * gpc], f32, tag="whh")
                nc.sync.dma_start(out=wih_k[:], in_=wihT[k * 128:(k + 1) * 128, :])
                nc.sync.dma_start(out=whh_k[:], in_=whhT[k * 128:(k + 1) * 128, :])
                for m in range(2 * MT):
                    nc.tensor.matmul(out=ps_rz[:, 2 * m:2 * m + 2],
                                     lhsT=wih_k[:, 128 * m:128 * m + 128],
                                     rhs=xg_sb[:, k, :],
                                     start=(k == 0 and m == 0), stop=False)
                    nc.tensor.matmul(out=ps_rz[:, 2 * m:2 * m + 2],
                                     lhsT=whh_k[:, 128 * m:128 * m + 128],
                                     rhs=hg_sb[:, k, :], start=False,
                                     stop=(k == KCH - 1 and m == 2 * MT - 1))
                for m in range(MT):
                    mm = 2 * MT + m
                    nc.tensor.matmul(out=ps_in[:, 2 * m:2 * m + 2],
                                     lhsT=wih_k[:, 128 * mm:128 * mm + 128],
                                     rhs=xg_sb[:, k, :],
                                     start=(k == 0 and m == 0),
                                     stop=(k == KCH - 1 and m == MT - 1))
                    nc.tensor.matmul(out=ps_hn[:, 2 * m:2 * m + 2],
                                     lhsT=whh_k[:, 128 * mm:128 * mm + 128],
                                     rhs=hg_sb[:, k, :],
                                     start=(k == 0 and m == 0),
                                     stop=(k == KCH - 1 and m == MT - 1))

            brz_sb = gru_sb_pool.tile([128, 2 * MT], f32)
            bnih_sb = gru_sb_pool.tile([128, MT], f32)
            bnhh_sb = gru_sb_pool.tile([128, MT], f32)
            hl_sb = gru_sb_pool.tile([128, MT, 2], f32)
            nc.sync.dma_start(out=brz_sb[:], in_=brz[:])
            nc.sync.dma_start(out=bnih_sb[:], in_=bnih[:])
            nc.sync.dma_start(out=bnhh_sb[:], in_=bnhh[:])
            nc.sync.dma_start(out=hl_sb[:], in_=hl[:])
            hp = gru_sb_pool.tile([128, MT, 2], f32)
            for m in range(MT):
                r_m = gru_sb_pool.tile([128, 2], f32, tag="r_m")
                z_m = gru_sb_pool.tile([128, 2], f32, tag="z_m")
                hn_m = gru_sb_pool.tile([128, 2], f32, tag="hn_m")
                nn_m = gru_sb_pool.tile([128, 2], f32, tag="nn_m")
                nc.scalar.activation(r_m[:], ps_rz[:, 2 * m:2 * m + 2],
                                     AF.Sigmoid, bias=brz_sb[:, m:m + 1], scale=1.0)
                zi = MT + m
                nc.scalar.activation(z_m[:], ps_rz[:, 2 * zi:2 * zi + 2],
                                     AF.Sigmoid, bias=brz_sb[:, zi:zi + 1], scale=1.0)
                nc.vector.tensor_add(hn_m[:], ps_hn[:, 2 * m:2 * m + 2],
                                     bnhh_sb[:, m:m + 1].to_broadcast([128, 2]))
                nc.vector.tensor_mul(hn_m[:], r_m[:], hn_m[:])
                nc.vector.tensor_add(hn_m[:], hn_m[:], ps_in[:, 2 * m:2 * m + 2])
                nc.scalar.activation(nn_m[:], hn_m[:],
                                     AF.Tanh, bias=bnih_sb[:, m:m + 1], scale=1.0)
                t1 = gru_sb_pool.tile([128, 2], f32, tag="t1")
                nc.vector.tensor_sub(t1[:], hl_sb[:, m, :], nn_m[:])
                nc.vector.tensor_mul(t1[:], z_m[:], t1[:])
                nc.vector.tensor_add(hp[:, m, :], nn_m[:], t1[:])
            for m in range(MT):
                nc.sync.dma_start(out=wbounce[128 * m:128 * m + 128, :],
                                  in_=hp[:, m, :])
            nc.gpsimd.collective_compute(
                "AllGather", mybir.AluOpType.bypass, replica_groups=RG,
                ins=[wbounce.ap().opt()], outs=[wfull.ap().opt()])
            w1_sb = const_pool.tile([64, D], f32)
            w2_sb = const_pool.tile([64, D], f32)
            wf3 = wfull.ap().rearrange("(a b) c -> a b c", b=D)
            nc.sync.dma_start(out=w1_sb[:], in_=wf3[:, :, 0])
            nc.sync.dma_start(out=w2_sb[:], in_=wf3[:, :, 1])
            b1_sb = const_pool.tile([64, 1], f32)
            b2_sb = const_pool.tile([64, 1], f32)
            nc.sync.dma_start(out=b1_sb[:], in_=gb1.ap().rearrange("a b -> b a"))
            nc.sync.dma_start(out=b2_sb[:], in_=gb2.ap().rearrange("a b -> b a"))

        # ---------------- x_scaled --------------------------------------
        with (
            tc.tile_pool(name="xsc", bufs=3) as x_pool,
            tc.tile_pool(name="normx", bufs=1) as normx_pool,
        ):
            dgx = normx_pool.tile([128, NT_X], f32)
            nc.sync.dma_start(out=dgx[:], in_=degoutx_t[:])
            onx = normx_pool.tile([128, NT_X], f32)
            nc.vector.reciprocal(onx[:], dgx[:])
            nc.scalar.activation(onx[:], onx[:], AF.Sqrt, scale=1.0)
            XT_P = 32
            full_tiles = n_nodes // 128
            for t0 in range(0, full_tiles, XT_P):
                t1_ = min(t0 + XT_P, full_tiles)
                k = t1_ - t0
                xt = x_pool.tile([128, XT_P, D], f32, tag="xt")
                nc.sync.dma_start(
                    out=xt[:, :k, :],
                    in_=emb[t0 * 128:t1_ * 128, :]
                    .rearrange("(a p) d -> p a d", p=128))
                xb = x_pool.tile([128, XT_P, D], f32, tag="xb")
                nc.vector.tensor_mul(xb[:, :k, :], xt[:, :k, :],
                                     onx[:, t0:t1_].to_broadcast([128, k, D]))
                nc.sync.dma_start(
                    out=xs[t0 * 128:t1_ * 128, :]
                    .rearrange("(a p) d -> p a d", p=128),
                    in_=xb[:, :k, :])
            rem = n_nodes - full_tiles * 128
            if rem:
                xt = x_pool.tile([128, XT_P, D], f32, tag="xt")
                nc.sync.dma_start(out=xt[:rem, 0, :], in_=emb[full_tiles * 128:, :])
                xb = x_pool.tile([128, XT_P, D], f32, tag="xb")
                nc.vector.tensor_mul(xb[:rem, 0, :], xt[:rem, 0, :],
                                     onx[:rem, full_tiles:full_tiles + 1]
                                     .to_broadcast([rem, 1, D]))
                nc.sync.dma_start(out=xs[full_tiles * 128:n_nodes, :],
                                  in_=xb[:rem, 0, :])

        # ---------------- GCN layers ------------------------------------
        with (
            tc.tile_pool(name="norms", bufs=1) as n_pool,
            tc.tile_pool(name="slots", bufs=2) as slot_pool,
            tc.tile_pool(name="meta", bufs=3) as meta_pool,
            tc.tile_pool(name="stp", bufs=3) as st_pool,
            tc.tile_pool(name="psA", bufs=2, space="PSUM") as psA,
            tc.tile_pool(name="psB", bufs=2, space="PSUM") as psB,
            tc.tile_pool(name="psC", bufs=2, space="PSUM") as psC,
            tc.tile_pool(name="epi", bufs=3) as epi_pool,
        ):
            dout = n_pool.tile([128, T * 4], f32)
            nc.sync.dma_start(out=dout[:], in_=degout_t[:])
            onrm = n_pool.tile([128, T * 4], f32)
            nc.vector.reciprocal(onrm[:], dout[:])
            nc.scalar.activation(onrm[:], onrm[:], AF.Sqrt, scale=1.0)
            scat_sb = n_pool.tile([128, T * 32], i16)
            nc.sync.dma_start(out=scat_sb[:], in_=scat_t[:])
            h1scat_sb = n_pool.tile([128, T * 32], i16)
            nc.sync.dma_start(out=h1scat_sb[:], in_=h1scat_t[:])
            relc_sb = n_pool.tile([128, C], f32)
            nc.sync.dma_start(out=relc_sb[:], in_=relc_t[:])
            zsb = n_pool.tile([128, 4, D], f32)
            nc.vector.memset(zsb[:].rearrange("p a d -> p (a d)"), 0.0)
            for t in range(T):
                nc.sync.dma_start(
                    out=h1b[t].ap().rearrange("(a p) d -> p a d", p=128),
                    in_=zsb[:])

            h1ball = nc.dram_tensor(f"h1ball{_rep}", [T * COLS_PER_TILE, D], f32)
            for layer in (0, 1):
                if layer == 1 and VARIANT["one_cc"]:
                    for tt in range(T):
                        nc.sync.dma_start(
                            out=h1ball[tt * COLS_PER_TILE:(tt + 1) * COLS_PER_TILE, :],
                            in_=h1b[tt][:])
                    nc.gpsimd.collective_compute(
                        "AllGather", mybir.AluOpType.bypass, replica_groups=RG,
                        ins=[h1ball.ap().opt()], outs=[h1s.ap().opt()])
                idx_tab = idx1_t if layer == 0 else idx2_t
                w_sb = w1_sb if layer == 0 else w2_sb
                for t in range(T):
                    slots = slot_pool.tile([128, CPT, D], f32, tag="slots")
                    if VARIANT["no_gather"]:
                        nc.vector.memset(
                            slots[:].rearrange("p a d -> p (a d)"), 0.0)
                    for kcl in range(4):
                        idx = meta_pool.tile([128, 256], i16, tag="idx")
                        nc.sync.dma_start(
                            out=idx[:],
                            in_=idx_tab[:, (t * 4 + kcl) * 256:(t * 4 + kcl + 1) * 256])
                        if layer == 0:
                            in_ap = (xs.ap()
                                     .rearrange("(a b) d -> a (b d)", b=4)
                                     [:, kcl * D:(kcl + 1) * D])
                        else:
                            in_ap = (h1s.ap()
                                     .rearrange("(a b) d -> a (b d)", b=4)
                                     [:, kcl * D:(kcl + 1) * D])
                        GI = VARIANT["g_idx"]
                        ncall = 4096 // GI
                        chpc = GI // 128          # chunks per call
                        for h in range(ncall):
                            if VARIANT["no_gather"]:
                                break
                            nc.gpsimd.dma_gather(
                                out_ap=slots[:, kcl * 32 + h * chpc:
                                             kcl * 32 + (h + 1) * chpc, :],
                                in_ap=in_ap,
                                idxs_ap=idx[:, h * (GI // 16):(h + 1) * (GI // 16)],
                                num_idxs=GI, num_idxs_reg=GI,
                                elem_size=D, elem_step=4 * D,
                                single_packet=VARIANT["sp"],
                                queue_num=(t * 4 + kcl) % VARIANT["g_q"])
                    stair = st_pool.tile([128, CPT, BIN_COLS], f32, tag="stair")
                    nc.vector.tensor_tensor(
                        out=stair[:],
                        in0=relc_sb[:, t * CPT:(t + 1) * CPT]
                        .to_broadcast([128, CPT, BIN_COLS]),
                        in1=bass.AP(iota_sb[:].tensor, iota_sb[:].offset,
                                    [iota_sb[:].ap[0], [0, CPT], iota_sb[:].ap[1]]),
                        op=mybir.AluOpType.is_equal)
                    agg_ps = psA.tile([64, COLS_PER_TILE], f32, tag="agg")
                    for q in ([0, CPT - 1] if VARIANT["no_pe"] else range(CPT)):
                        g = q % BINS_PER_TILE
                        kcl = q // BINS_PER_TILE
                        nc.tensor.matmul(
                            out=agg_ps[:, g * BIN_COLS:(g + 1) * BIN_COLS],
                            lhsT=slots[:, kcl * 32 + g, :],
                            rhs=stair[:, kcl * 32 + g, :],
                            start=(q == 0), stop=(q == CPT - 1))
                    # replicate this tile's in-degree row across partitions,
                    # rsqrt on device, then scale the aggregate
                    dint = meta_pool.tile([64, COLS_PER_TILE], f32, tag="dint")
                    nc.sync.dma_start(
                        out=dint[:],
                        in_=bass.AP(degin_t.ap().tensor, t * COLS_PER_TILE,
                                    [[0, 64], [1, COLS_PER_TILE]]))
                    inrm_t = meta_pool.tile([64, COLS_PER_TILE], f32, tag="inrm")
                    nc.vector.reciprocal(inrm_t[:], dint[:])
                    nc.scalar.activation(inrm_t[:], inrm_t[:], AF.Sqrt, scale=1.0)
                    aggs = epi_pool.tile([64, COLS_PER_TILE], f32, tag="aggs")
                    nc.vector.tensor_mul(aggs[:], agg_ps[:], inrm_t[:])
                    h_ps = psB.tile([64, COLS_PER_TILE], f32, tag="h")
                    nc.tensor.matmul(out=h_ps[:], lhsT=w_sb[:], rhs=aggs[:],
                                     start=True, stop=True)
                    hb = epi_pool.tile([64, COLS_PER_TILE], f32, tag="hb")
                    if layer == 0:
                        nc.scalar.activation(hb[:], h_ps[:], AF.Relu,
                                             bias=b1_sb[:], scale=1.0)
                    else:
                        nc.vector.tensor_add(
                            hb[:], h_ps[:],
                            b2_sb[:].to_broadcast([64, COLS_PER_TILE]))
                    hn = epi_pool.tile([128, 4, D], f32, tag="hn")
                    for b in range(4):
                        tp_ps = psC.tile([128, D], f32, tag="tp")
                        nc.tensor.transpose(out=tp_ps[:],
                                            in_=hb[:, 128 * b:128 * b + 128],
                                            identity=ident[:])
                        if layer == 0:
                            nc.vector.tensor_mul(
                                hn[:, b, :], tp_ps[:],
                                onrm[:, t * 4 + b:t * 4 + b + 1]
                                .to_broadcast([128, D]))
                        else:
                            nc.vector.tensor_copy(hn[:, b, :], tp_ps[:])
                    if layer == 0:
                        nc.gpsimd.dma_scatter_add(
                            out_ap=h1b[t][:], in_ap=hn[:],
                            idxs_ap=h1scat_sb[:, t * 32:(t + 1) * 32],
                            num_idxs=COLS_PER_TILE, num_idxs_reg=COLS_PER_TILE,
                            elem_size=D)
                        if VARIANT["no_cc"] or VARIANT["one_cc"]:
                            nc.sync.dma_start(
                                out=h1s[t * N_CORES * COLS_PER_TILE:
                                        t * N_CORES * COLS_PER_TILE
                                        + COLS_PER_TILE, :],
                                in_=h1b[t][:])
                        else:
                            nc.gpsimd.collective_compute(
                                "AllGather", mybir.AluOpType.bypass,
                                replica_groups=RG,
                                ins=[h1b[t].ap().opt()],
                                outs=[h1s[t * N_CORES * COLS_PER_TILE:
                                          (t + 1) * N_CORES * COLS_PER_TILE, :].opt()])
                    else:
                        nc.gpsimd.dma_scatter_add(
                            out_ap=out_t[:], in_ap=hn[:],
                            idxs_ap=scat_sb[:, t * 32:(t + 1) * 32],
                            num_idxs=COLS_PER_TILE, num_idxs_reg=COLS_PER_TILE,
                            elem_size=D)
    nc.compile()
    return nc


# ------------------------------------------------------------------- driver
def make_in_maps(inputs, P, n_nodes, hdim):
    gpc = hdim // N_CORES
    KCH = hdim // 128
    MT = gpc // 128
    T = P["T"]

    X = np.stack([np.asarray(inputs["prev_gc1"]), np.asarray(inputs["prev_gc2"])], 1)
    Hm = np.stack([np.asarray(inputs["gc1_weight"]).reshape(-1),
                   np.asarray(inputs["gc2_weight"]).reshape(-1)], 1)
    xg_d = np.ascontiguousarray(X.reshape(KCH, 128, 2).transpose(1, 0, 2), np.float32)
    hg_d = np.ascontiguousarray(Hm.reshape(KCH, 128, 2).transpose(1, 0, 2), np.float32)

    W_ih = np.asarray(inputs["W_ih"]); W_hh = np.asarray(inputs["W_hh"])
    b_ih = np.asarray(inputs["b_ih"]); b_hh = np.asarray(inputs["b_hh"])
    emb = np.ascontiguousarray(np.asarray(inputs["node_embeddings"], np.float32))
    iota = np.tile(np.arange(BIN_COLS, dtype=np.float32), (128, 1))

    in_maps = []
    for c in range(N_CORES):
        rows = np.concatenate([np.arange(g * hdim + c * gpc, g * hdim + (c + 1) * gpc)
                               for g in range(3)])
        wihT_c = np.ascontiguousarray(W_ih[rows].T, np.float32)
        whhT_c = np.ascontiguousarray(W_hh[rows].T, np.float32)
        brz_c = np.ascontiguousarray(
            (b_ih[rows] + b_hh[rows])[:2 * gpc].reshape(2 * MT, 128).T, np.float32)
        bnih_c = np.ascontiguousarray(
            b_ih[rows][2 * gpc:].reshape(MT, 128).T, np.float32)
        bnhh_c = np.ascontiguousarray(
            b_hh[rows][2 * gpc:].reshape(MT, 128).T, np.float32)
        hl_c = np.ascontiguousarray(
            Hm[c * gpc:(c + 1) * gpc].reshape(MT, 128, 2).transpose(1, 0, 2),
            np.float32)
        core = P["cores"][c]
        in_maps.append({
            "emb": emb, "wihT": wihT_c, "whhT": whhT_c,
            "xg": xg_d, "hg": hg_d, "hl": hl_c,
            "brz": brz_c, "bnih": bnih_c, "bnhh": bnhh_c,
            "gb1": np.asarray(inputs["gc1_bias"], np.float32).reshape(1, D),
            "gb2": np.asarray(inputs["gc2_bias"], np.float32).reshape(1, D),
            "idx1": np.ascontiguousarray(core["idx1"]),
            "idx2": np.ascontiguousarray(core["idx2"]),
            "relc": core["relc"], "iota": iota,
            "scat": np.ascontiguousarray(core["scat"]),
            "h1scat": np.ascontiguousarray(core["h1scat"]),
            "degin": core["deg_in_row"], "degout": core["deg_out_blk"],
            "degoutx": P["deg_out_x"],
        })
    return in_maps


def kernel(node_embeddings, gc1_weight, gc2_weight, gc1_bias, gc2_bias,
           prev_gc1, prev_gc2, W_ih, W_hh, b_ih, b_hh, src, dst):
    from concourse.bass_utils import run_bass_kernel_spmd

    inputs = dict(node_embeddings=node_embeddings, gc1_weight=gc1_weight,
                  gc2_weight=gc2_weight, gc1_bias=gc1_bias, gc2_bias=gc2_bias,
                  prev_gc1=prev_gc1, prev_gc2=prev_gc2, W_ih=W_ih, W_hh=W_hh,
                  b_ih=b_ih, b_hh=b_hh, src=src, dst=dst)
    n_nodes = np.asarray(node_embeddings).shape[0]
    npc = n_nodes // N_CORES
    hdim = np.asarray(prev_gc1).shape[0]
    src = np.asarray(src); dst = np.asarray(dst)

    P = preprocess(src, dst, n_nodes)
    nc = build_kernel(n_nodes, P["T"], P["NT_X"], hdim)
    in_maps = make_in_maps(inputs, P, n_nodes, hdim)
    res = run_bass_kernel_spmd(nc, in_maps, core_ids=list(range(N_CORES)))
    out = np.concatenate([res.results[c]["out"][:npc] for c in range(N_CORES)], 0)
    return out.astype(np.float32)

